# revision 47
# baseline (speedup 1.0000x reference)
"""DeltaNet decode step on 8 Trainium2 NeuronCores (tensor-parallel over heads).

Contract: kernel(**inputs) takes the FULL unsharded inputs (numpy arrays,
same keys as the reference setup_inputs()) and returns the FULL output
[1, 4096, 1, 1] float32.

Sharding (8 cores, 16 heads -> 2 heads/core):
  - Wq/Wk rows, q/k conv weights+caches: 512 rows per core
  - Wv rows, v conv weights+caches, Wo columns: 1024 per core
  - state: 2 heads per core
  - output: each core computes a partial [4096] projection; host all-reduces.

Device kernel: memory-bound mat-vec streaming. Precision per stream is
chosen against the 2e-2 rel-err gate (end-to-end error of this scheme is
~8e-3): Wq/Wv/Wo single bf16, Wk fp8-e4m3 (scaled by 2048 on host; h for
the k-matvec is fp8 scaled by 32; the 1/65536 unscale is folded into the
k-conv tap-3 weights), state bf16. Weights are pre-packed on host into
contiguous DRAM images laid out exactly as the SBUF tiles consume them.

DMA: one HWDGE ring (sync) streams the weight tiles (splitting over both
rings reaches ~390-430 GB/s but trips device power throttling that
clamps the PE — single-ring ~340 GB/s is the measured optimum). hb/wabt
ride the same ring ahead of the stream so the PE warmup + alpha/beta
matvec start right after the engine preamble; the first weight tiles are
1 MB so real matmuls start early. The last Wo MB is split in two 0.5 MB
pieces, the final PSUM->SBUF copies are interleaved with the closing
matmuls, and the out DMA goes in two halves to shorten the tail.

The post-matvec chain (conv+silu, l2norm, state update, combine) runs in
128-lane column layout injected into the Wv streaming phase so it stays
off the DMA critical path; row->column moves use K=1 outer-product
matmuls (lhsT=[1,128] row slice, rhs=[1,1] const 1.0).
"""

import os
import sys
import types

sys.path.insert(0, "/opt/trn_rl_repo")

import numpy as np
import ml_dtypes

import concourse.bass as bass
import concourse.mybir as mybir
import concourse.tile as tile
from concourse import bacc
from concourse.bass_utils import run_bass_kernel_spmd

BF16 = ml_dtypes.bfloat16
FP8 = ml_dtypes.float8_e4m3     # what mybir.dt.float8e4 maps to (max 240)
F32 = mybir.dt.float32
BF = mybir.dt.bfloat16
F8 = mybir.dt.float8e4
AF = mybir.ActivationFunctionType
OP = mybir.AluOpType

H = 4096
QK = 4096
VD = 8192
EPS = 1e-6
NCORES = 8
HPC = 2          # heads per core
RQ = 512         # q/k rows per core
RV = 1024        # v rows / Wo cols per core
SK = 2048.0      # fp8 scale for Wk (|Wk|max ~0.11 -> ~222 < 240)
SH = 32.0        # fp8 scale for h  (|h|max  ~4.2  -> ~134 < 240)

_CACHE = {}


def _ensure_ntff_hook():
    """Install the axon NTFF profile hook shim (antenv.axon_hooks is absent
    in this image). Harmless if profiling is never requested."""
    if "antenv.axon_hooks" in sys.modules:
        return
    try:
        import antenv
        mod = types.ModuleType("antenv.axon_hooks")
        mod._hook = None
        mod.set_axon_ntff_profile_hook = lambda h: setattr(mod, "_hook", h)
        mod.get_axon_ntff_profile_hook = lambda: mod._hook
        sys.modules["antenv.axon_hooks"] = mod
        antenv.axon_hooks = mod
        from trn_agent_boot.trn_boot import _ntff_profile_via_ctypes
        mod._hook = _ntff_profile_via_ctypes("/opt/axon/libaxon_pjrt.so")
    except Exception:
        pass


def _build_nc():
    nc = bacc.Bacc(None)

    d = {}
    d["wk_img"] = nc.dram_tensor("wk_img", [128, 16384], F8, kind="ExternalInput")
    d["wq_img"] = nc.dram_tensor("wq_img", [128, 16384], BF, kind="ExternalInput")
    d["wv_img"] = nc.dram_tensor("wv_img", [128, 32768], BF, kind="ExternalInput")
    d["wo_img"] = nc.dram_tensor("wo_img", [128, 32768], BF, kind="ExternalInput")
    d["hw_sm"] = nc.dram_tensor("hw_sm", [128, 160], BF, kind="ExternalInput")
    d["state_c"] = nc.dram_tensor("state_c", [128, 2048], BF, kind="ExternalInput")
    d["qkcache"] = nc.dram_tensor("qkcache", [128, 24], F32, kind="ExternalInput")
    d["qkconvw"] = nc.dram_tensor("qkconvw", [128, 32], F32, kind="ExternalInput")
    d["vcache"] = nc.dram_tensor("vcache", [128, 24], F32, kind="ExternalInput")
    d["vconvw"] = nc.dram_tensor("vconvw", [128, 32], F32, kind="ExternalInput")
    out_d = nc.dram_tensor("out", [1, H], F32, kind="ExternalOutput")

    with tile.TileContext(nc) as tc:
        with (
            tc.tile_pool(name="smalls", bufs=1) as sm,
            tc.tile_pool(name="wpool", bufs=8) as wp,
            tc.tile_pool(name="psum", bufs=8, space="PSUM") as pm,
        ):
            def emit():
                # ---- small input DMAs. hb+wab ride the fast HWDGE ring in
                # one packed DMA ahead of the weight stream so the PE warmup
                # + ab matvec can start right after the engine preamble
                # (SWDGE smalls take ~10us to land). hb8 (fp8 h, scaled) is
                # derived on-device. The rest stays on SWDGE.
                # cols 0-31 = h (bf16, column layout), cols 32-159 = wab
                hw_sm = sm.tile([128, 160], BF, tag="hw_sm")
                nc.sync.dma_start(out=hw_sm[:], in_=d["hw_sm"][:])
                hb8 = sm.tile([128, 32], F8, tag="hb8")
                nc.scalar.activation(hb8[:], hw_sm[:, 0:32], AF.Copy, scale=SH)
                st = sm.tile([128, 2048], BF, tag="st")
                qkca = sm.tile([128, 24], F32, tag="qkca")
                qkcw = sm.tile([128, 32], F32, tag="qkcw")
                vca = sm.tile([128, 24], F32, tag="vca")
                vcw = sm.tile([128, 32], F32, tag="vcw")
                for t, src in [(st, "state_c"), (qkca, "qkcache"),
                               (qkcw, "qkconvw"), (vca, "vcache"),
                               (vcw, "vconvw")]:
                    nc.gpsimd.dma_start(out=t[:], in_=d[src][:])
                ones = sm.tile([1, 128], F32, tag="ones")
                nc.vector.memset(ones[:], 1.0)
                ones_b = sm.tile([1, 128], BF, tag="ones_b")
                nc.vector.memset(ones_b[:], 1.0)
                onesc = sm.tile([128, 1], F32, tag="onesc")
                nc.vector.memset(onesc[:], 1.0)
                epst = sm.tile([1, 1], F32, tag="epst")
                nc.vector.memset(epst[:], EPS)
                wz = sm.tile([128, 512], BF, tag="wz")
                nc.vector.memset(wz[:], 0.0)

                # ---- PE warmup: ramp the p-state before the stream lands ----
                ps_wm = pm.tile([1, 512], F32, tag="ps")
                for r in range(16):
                    nc.tensor.matmul(ps_wm[0:1, :], hw_sm[:, 0:1], wz[:],
                                     start=True, stop=True)

                # ---- big weight tile stream, single HWDGE ring ----
                # 12 DMAs on nc.sync. First 8 are pre-issued (fills the
                # pool); the rest are issued right after an early tile's
                # matmuls so the ring FIFO never head-of-line blocks.
                # (Measured: splitting the stream over both HWDGE rings
                # pushes the burst rate to ~390-430 GB/s but triggers power
                # throttling that clamps the PE to ~1.35 GHz — net loss.
                # A single ring at ~340 GB/s is the equilibrium optimum.)
                tiles = []

                def issue_tile(src_ap, width, dt):
                    t = wp.tile([128, width], dt, tag="w", name="wtile")
                    nc.sync.dma_start(out=t[:], in_=src_ap)
                    tiles.append(t)

                issue_tile(d["wk_img"][:], 16384, F8)               # 0: k (2M)
                issue_tile(d["wq_img"][:, 0:8192], 8192, BF)        # 1: q 0-15
                issue_tile(d["wq_img"][:, 8192:16384], 8192, BF)    # 2: q 16-31
                for vv in range(4):                                 # 3-6: v 2M
                    issue_tile(d["wv_img"][:, 8192 * vv:8192 * vv + 8192],
                               8192, BF)
                issue_tile(d["wo_img"][:, 0:8192], 8192, BF)        # 7: o j0,j1
                late = [lambda: issue_tile(d["wo_img"][:, 8192:16384], 8192, BF),
                        lambda: issue_tile(d["wo_img"][:, 16384:24576], 8192, BF),
                        lambda: issue_tile(d["wo_img"][:, 24576:28672], 4096, BF),
                        lambda: issue_tile(d["wo_img"][:, 28672:32768], 4096, BF)]

                # ---- alpha/beta matvec (bf16, tiny; also keeps PE warm
                # until the first weight tiles land) ----
                ps_ab = pm.tile([1, 4], F32, tag="ps")
                for cc in range(32):
                    nc.tensor.matmul(
                        ps_ab[0:1, 0:4], hw_sm[:, cc:cc + 1],
                        hw_sm[:, 32 + 4 * cc:36 + 4 * cc],
                        start=(cc == 0), stop=(cc == 31))
                ab = sm.tile([1, 4], F32, tag="ab")
                nc.scalar.activation(ab[:], ps_ab[:], AF.Sigmoid)

                # ---- k matvec (fp8 x fp8, result scaled by SK*SH) ----
                ps_k = pm.tile([1, 512], F32, tag="ps")
                t = tiles[0]
                for cc in range(32):
                    nc.tensor.matmul(
                        ps_k[0:1, :], hb8[:, cc:cc + 1],
                        t[:, 512 * cc:512 * cc + 512],
                        start=(cc == 0), stop=(cc == 31))
                late[0]()

                # ---- q matvec (bf16) ----
                ps_q = pm.tile([1, 512], F32, tag="ps")
                for qt in range(2):
                    t = tiles[1 + qt]
                    for i in range(16):
                        cc = 16 * qt + i
                        nc.tensor.matmul(
                            ps_q[0:1, :], hw_sm[:, cc:cc + 1],
                            t[:, 512 * i:512 * i + 512],
                            start=(cc == 0), stop=(cc == 31))
                    late[1 + qt]()
                qrow = sm.tile([1, 512], BF, tag="qrow")
                nc.vector.tensor_copy(qrow[:], ps_q[0:1, :])
                krow = sm.tile([1, 512], BF, tag="krow")
                nc.vector.tensor_copy(krow[:], ps_k[0:1, :])

                # The rest of the q/k chain runs in 128-lane column layout
                # (cols 0-3 = k chunks, 4-7 = q chunks); the per-head
                # reductions (l2norm sum-sq, q.k dot) use ones-column fp32
                # matmuls for the partition-dim sum. All PE pieces are
                # injected into the Wv streaming phase to fill DMA-wait gaps.
                t_qk = pm.tile([128, 8], F32, tag="ps")
                qkcol = sm.tile([128, 8], F32, tag="qkcol")
                qacc = sm.tile([128, 8], F32, tag="qacc")
                qtmp = sm.tile([128, 8], F32, tag="qtmp")
                x1 = sm.tile([128, 8], F32, tag="x1")
                sq = sm.tile([128, 8], F32, tag="sq")
                ps_ss = pm.tile([1, 8], F32, tag="ps")
                ssr = sm.tile([1, 8], F32, tag="ssr")
                ssh = sm.tile([1, 4], F32, tag="ssh")
                srt = sm.tile([1, 4], F32, tag="srt")
                rin = sm.tile([1, 4], F32, tag="rin")
                t_rn = pm.tile([128, 4], F32, tag="ps")
                rbc = sm.tile([128, 4], F32, tag="rbc")
                qkn = sm.tile([128, 8], F32, tag="qkn")
                qkn_b = sm.tile([128, 8], BF, tag="qkn_b")
                dm = sm.tile([128, 4], F32, tag="dm")
                ps_dot = pm.tile([1, 4], F32, tag="ps")
                dotr = sm.tile([1, 4], F32, tag="dotr")
                dot = sm.tile([1, 2], F32, tag="dot")
                bd = sm.tile([1, 2], F32, tag="bd")
                t_bc = pm.tile([128, 4], F32, tag="ps")
                abc = sm.tile([128, 4], F32, tag="abc")
                ps_stc = pm.tile([128, 16], F32, tag="ps")

                def chain_pe_0():
                    # raw q/k rows -> columns (K=1 outer products, bf16)
                    for c in range(4):
                        nc.tensor.matmul(t_qk[:, c:c + 1],
                                         krow[0:1, 128 * c:128 * c + 128],
                                         ones_b[0:1, 0:1], start=True, stop=True)
                        nc.tensor.matmul(t_qk[:, 4 + c:5 + c],
                                         qrow[0:1, 128 * c:128 * c + 128],
                                         ones_b[0:1, 0:1], start=True, stop=True)
                    nc.vector.tensor_copy(qkcol[:], t_qk[:])
                    # conv + silu in columns (k tap-3 weights carry 1/(SK*SH))
                    nc.vector.tensor_mul(qacc[:], qkca[:, 0:8], qkcw[:, 0:8])
                    for tpi in (1, 2):
                        nc.vector.tensor_mul(qtmp[:], qkca[:, 8 * tpi:8 * tpi + 8],
                                             qkcw[:, 8 * tpi:8 * tpi + 8])
                        nc.vector.tensor_add(qacc[:], qacc[:], qtmp[:])
                    nc.vector.tensor_mul(qtmp[:], qkcol[:], qkcw[:, 24:32])
                    nc.vector.tensor_add(qacc[:], qacc[:], qtmp[:])
                    nc.scalar.activation(x1[:], qacc[:], AF.Sigmoid)
                    nc.vector.tensor_mul(x1[:], qacc[:], x1[:])
                    nc.vector.tensor_mul(sq[:], x1[:], x1[:])

                def chain_pe_1():
                    # per-column sum of squares, then per-head l2 scale
                    nc.tensor.matmul(ps_ss[0:1, :], onesc[:, 0:1], sq[:],
                                     start=True, stop=True)
                    nc.vector.tensor_copy(ssr[:], ps_ss[0:1, :])
                    nc.vector.reduce_sum(
                        ssh[0:1, 0:4],
                        ssr[0:1, :].rearrange("a (g t) -> a g t", t=2),
                        axis=mybir.AxisListType.X)
                    nc.scalar.activation(srt[:], ssh[:], AF.Sqrt,
                                         bias=epst[0:1, 0:1])
                    nc.vector.reciprocal(rin[:], srt[:])

                def chain_pe_2():
                    # broadcast 1/norm, normalize columns
                    rin_b = sm.tile([1, 4], BF, tag="rin_b")
                    nc.vector.tensor_copy(rin_b[:], rin[:])
                    for j in range(4):
                        nc.tensor.matmul(t_rn[:, j:j + 1], ones_b[0:1, :],
                                         rin_b[0:1, j:j + 1], start=True, stop=True)
                    nc.vector.tensor_copy(rbc[:], t_rn[:])
                    for g in range(4):  # k_h0, k_h1, q_h0, q_h1 col pairs
                        nc.vector.tensor_scalar(
                            out=qkn[:, 2 * g:2 * g + 2],
                            in0=x1[:, 2 * g:2 * g + 2],
                            scalar1=rbc[:, g:g + 1], scalar2=None, op0=OP.mult)
                    nc.vector.tensor_copy(qkn_b[:], qkn[:])
                    # q.k dot per head
                    nc.vector.tensor_mul(dm[:], qkn[:, 4:8], qkn[:, 0:4])
                    nc.tensor.matmul(ps_dot[0:1, :], onesc[:, 0:1], dm[:],
                                     start=True, stop=True)
                    nc.vector.tensor_copy(dotr[:], ps_dot[0:1, :])
                    nc.vector.reduce_sum(
                        dot[0:1, 0:2],
                        dotr[0:1, :].rearrange("a (g t) -> a g t", t=2),
                        axis=mybir.AxisListType.X)
                    nc.vector.tensor_mul(bd[:], ab[0:1, 2:4], dot[0:1, 0:2])
                    # broadcast alpha / beta*dot to partitions (bf16)
                    ab_b = sm.tile([1, 4], BF, tag="ab_b")
                    nc.vector.tensor_copy(ab_b[:], ab[:])
                    bd_b = sm.tile([1, 2], BF, tag="bd_b")
                    nc.vector.tensor_copy(bd_b[:], bd[:])
                    for hh in range(HPC):
                        nc.tensor.matmul(t_bc[:, hh:hh + 1], ones_b[0:1, :],
                                         ab_b[0:1, hh:hh + 1],
                                         start=True, stop=True)
                        nc.tensor.matmul(t_bc[:, 2 + hh:3 + hh], ones_b[0:1, :],
                                         bd_b[0:1, hh:hh + 1],
                                         start=True, stop=True)
                    nc.vector.tensor_copy(abc[:], t_bc[:])
                    # state matvecs (bf16 state, column outputs)
                    for hh in range(HPC):
                        for which in range(2):  # 0 -> k, 1 -> q
                            for vc in range(4):
                                col = 8 * which + 4 * hh + vc
                                for d2 in range(2):
                                    blk = 2 * hh + d2
                                    nc.tensor.matmul(
                                        ps_stc[:, col:col + 1],
                                        st[:, 512 * blk + 128 * vc:
                                           512 * blk + 128 * vc + 128],
                                        qkn_b[:, 4 * which + 2 * hh + d2:
                                              4 * which + 2 * hh + d2 + 1],
                                        start=(d2 == 0), stop=(d2 == 1))

                # ---- v matvec, with chain injected into DMA-wait gaps ----
                ps_v0 = pm.tile([1, 512], F32, tag="ps")
                ps_v1 = pm.tile([1, 512], F32, tag="ps")
                inject = {0: chain_pe_0, 1: chain_pe_1, 2: chain_pe_2}
                for dd in range(4):
                    t = tiles[3 + dd]
                    for i in range(8):
                        cc = 8 * dd + i
                        nc.tensor.matmul(
                            ps_v0[0:1, :], hw_sm[:, cc:cc + 1],
                            t[:, 1024 * i:1024 * i + 512],
                            start=(cc == 0), stop=(cc == 31))
                        nc.tensor.matmul(
                            ps_v1[0:1, :], hw_sm[:, cc:cc + 1],
                            t[:, 1024 * i + 512:1024 * i + 1024],
                            start=(cc == 0), stop=(cc == 31))
                    if dd == 0:
                        late[3]()
                    if dd in inject:
                        inject[dd]()

                vsb = sm.tile([1, 1024], BF, tag="vsb")
                nc.vector.tensor_copy(vsb[0:1, 0:512], ps_v0[0:1, :])
                nc.vector.tensor_copy(vsb[0:1, 512:1024], ps_v1[0:1, :])
                # transpose v row to columns: vcol[p, j] = vsb[0, 128j+p]
                t_v = pm.tile([128, 8], F32, tag="ps")
                for j in range(8):
                    nc.tensor.matmul(t_v[:, j:j + 1],
                                     vsb[0:1, 128 * j:128 * j + 128],
                                     ones_b[0:1, 0:1], start=True, stop=True)
                vcol = sm.tile([128, 8], F32, tag="vcol")
                nc.vector.tensor_copy(vcol[:], t_v[:])

                # ---- v conv + silu in columns [128, 8] ----
                vacc = sm.tile([128, 8], F32, tag="vacc")
                vtmp = sm.tile([128, 8], F32, tag="vtmp")
                nc.vector.tensor_mul(vacc[:], vca[:, 0:8], vcw[:, 0:8])
                for tpi in (1, 2):
                    nc.vector.tensor_mul(vtmp[:], vca[:, 8 * tpi:8 * tpi + 8],
                                         vcw[:, 8 * tpi:8 * tpi + 8])
                    nc.vector.tensor_add(vacc[:], vacc[:], vtmp[:])
                nc.vector.tensor_mul(vtmp[:], vcol[:], vcw[:, 24:32])
                nc.vector.tensor_add(vacc[:], vacc[:], vtmp[:])
                v1c = sm.tile([128, 8], F32, tag="v1c")
                nc.scalar.activation(v1c[:], vacc[:], AF.Sigmoid)
                nc.vector.tensor_mul(v1c[:], vacc[:], v1c[:])

                # ---- combine in columns: ov = a*qs + (b*dot)*(v - a*ks) ----
                ovc = sm.tile([128, 8], F32, tag="ovc")
                errc = sm.tile([128, 4], F32, tag="errc")
                t1c = sm.tile([128, 4], F32, tag="t1c")
                for hh in range(HPC):
                    ks = ps_stc[:, 4 * hh:4 * hh + 4]
                    qs = ps_stc[:, 8 + 4 * hh:8 + 4 * hh + 4]
                    nc.vector.tensor_scalar(out=errc[:], in0=ks,
                                            scalar1=abc[:, hh:hh + 1],
                                            scalar2=None, op0=OP.mult)
                    nc.vector.tensor_sub(errc[:], v1c[:, 4 * hh:4 * hh + 4], errc[:])
                    nc.vector.tensor_scalar(out=t1c[:], in0=qs,
                                            scalar1=abc[:, hh:hh + 1],
                                            scalar2=None, op0=OP.mult)
                    nc.vector.tensor_scalar(out=errc[:], in0=errc[:],
                                            scalar1=abc[:, 2 + hh:3 + hh],
                                            scalar2=None, op0=OP.mult)
                    nc.vector.tensor_add(ovc[:, 4 * hh:4 * hh + 4], t1c[:], errc[:])

                # ---- ov to bf16 columns ----
                ov_b = sm.tile([128, 8], BF, tag="ov_b")
                nc.vector.tensor_copy(ov_b[:], ovc[:])

                # ---- output projection ----
                ps_o = [pm.tile([1, 512], F32, tag="ps", name=f"ps_o{i}")
                        for i in range(8)]
                out_sb = sm.tile([1, H], F32, tag="out_sb")
                for dd in range(3):     # tiles[7+dd]: j pairs (2MB each)
                    t = tiles[7 + dd]
                    for i in range(2):
                        j = 2 * dd + i
                        for it in range(8):
                            nc.tensor.matmul(
                                ps_o[it][0:1, :], ov_b[:, j:j + 1],
                                t[:, 4096 * i + 512 * it:4096 * i + 512 * it + 512],
                                start=(j == 0), stop=False)
                for it in range(8):     # tiles[10]: j = 6 (1MB)
                    nc.tensor.matmul(
                        ps_o[it][0:1, :], ov_b[:, 6:7],
                        tiles[10][:, 512 * it:512 * it + 512],
                        start=False, stop=False)
                # tiles[11]: j = 7 (1MB); copy each strip as it closes, and
                # start the out DMA per half as soon as it's ready
                for it in range(8):
                    nc.tensor.matmul(
                        ps_o[it][0:1, :], ov_b[:, 7:8],
                        tiles[11][:, 512 * it:512 * it + 512],
                        start=False, stop=True)
                    dst = out_sb[0:1, 512 * it:512 * it + 512]
                    if it % 2 == 0:
                        nc.vector.tensor_copy(dst, ps_o[it][0:1, :])
                    else:
                        nc.scalar.copy(dst, ps_o[it][0:1, :])
                    if it == 3 or it == 7:
                        half = it // 4
                        nc.sync.dma_start(
                            out=out_d[:, 2048 * half:2048 * half + 2048],
                            in_=out_sb[0:1, 2048 * half:2048 * half + 2048])

            emit()

    nc.finalize()
    return nc


def _prep_in_maps(inputs):
    f32 = np.float32
    hid = np.asarray(inputs["hidden_states"], f32)[0, :, 0, 0]     # [4096]
    Wq = np.asarray(inputs["Wq"], f32)
    Wk = np.asarray(inputs["Wk"], f32)
    Wv = np.asarray(inputs["Wv"], f32)
    Wo = np.asarray(inputs["Wo"], f32)
    Wa = np.asarray(inputs["Wa"], f32)
    Wb = np.asarray(inputs["Wb"], f32)
    qcw = np.asarray(inputs["q_conv_w"], f32)[0]                   # [QK, 4]
    kcw = np.asarray(inputs["k_conv_w"], f32)[0]
    vcw = np.asarray(inputs["v_conv_w"], f32)[0]                   # [VD, 4]
    qca = np.asarray(inputs["q_cache"], f32)[0]                    # [QK, 3]
    kca = np.asarray(inputs["k_cache"], f32)[0]
    vca = np.asarray(inputs["v_cache"], f32)[0]                    # [VD, 3]
    state = np.asarray(inputs["state"], f32)[0]                    # [16,256,512]

    # h in column layout [128, 32]: bf16 and fp8 (scaled) variants
    hcols = hid.reshape(32, 128).T
    hb_c = np.ascontiguousarray(hcols.astype(BF16))

    def chunk_img(wt, width):
        # wt [128*n, width] -> [128, n*width]: img[p, width*cc+r] = wt[128cc+p, r]
        n = wt.shape[0] // 128
        return np.ascontiguousarray(
            wt.reshape(n, 128, width).transpose(1, 0, 2).reshape(128, -1))

    in_maps = []
    for c in range(NCORES):
        rq = slice(c * RQ, (c + 1) * RQ)
        rv = slice(c * RV, (c + 1) * RV)
        wq_img = chunk_img(Wq[rq].T.astype(BF16), 512)
        wk_img = chunk_img((Wk[rq].T * SK).astype(FP8), 512)
        wv_img = chunk_img(Wv[rv].T.astype(BF16), 1024)            # [H,1024]->
        wo_img = chunk_img(Wo[:, rv].T.astype(BF16), 4096)         # j-major

        wab = np.concatenate([Wa[2 * c:2 * c + 2], Wb[2 * c:2 * c + 2]], 0)
        wab_sb = wab.reshape(4, 32, 128).transpose(2, 1, 0).reshape(128, 128)
        hw_sm = np.ascontiguousarray(np.concatenate(
            [hb_c, wab_sb.astype(BF16)], axis=1))                  # [128, 160]
        st_sb = np.ascontiguousarray(
            state[2 * c:2 * c + 2].reshape(2, 2, 128, 512)
            .transpose(2, 0, 1, 3).reshape(128, 2048).astype(BF16))

        # q/k conv in column layout [128, 8*taps]: per tap, cols 0-3 = k
        # chunks (k idx 128c+p), cols 4-7 = q chunks. The k tap-3 weights
        # absorb the fp8 matvec scale 1/(SK*SH).
        qk_ca = np.concatenate(
            [np.concatenate([kca[rq, t].reshape(4, 128).T,
                             qca[rq, t].reshape(4, 128).T], 1)
             for t in range(3)], 1)
        kcw_t = [kcw[rq, t] / ((SK * SH) if t == 3 else 1.0) for t in range(4)]
        qk_cw = np.concatenate(
            [np.concatenate([kcw_t[t].reshape(4, 128).T,
                             qcw[rq, t].reshape(4, 128).T], 1)
             for t in range(4)], 1)
        # v conv in column layout [128, 8*taps]: vcol[p, 8t+cc] = v[128cc+p, t]
        v_ca = np.ascontiguousarray(
            vca[rv].reshape(8, 128, 3).transpose(1, 2, 0).reshape(128, 24))
        v_cw = np.ascontiguousarray(
            vcw[rv].reshape(8, 128, 4).transpose(1, 2, 0).reshape(128, 32))

        in_maps.append({
            "wq_img": wq_img, "wk_img": wk_img,
            "wv_img": wv_img, "wo_img": wo_img,
            "hw_sm": hw_sm, "state_c": st_sb,
            "qkcache": np.ascontiguousarray(qk_ca),
            "qkconvw": np.ascontiguousarray(qk_cw),
            "vcache": v_ca, "vconvw": v_cw,
        })
    return in_maps


def _run(inputs, trace=False, tmpdir=None):
    _ensure_ntff_hook()
    if "nc" not in _CACHE:
        _CACHE["nc"] = _build_nc()
    nc = _CACHE["nc"]
    in_maps = _prep_in_maps(inputs)
    res = run_bass_kernel_spmd(nc, in_maps, list(range(NCORES)),
                               trace=trace, tmpdir=tmpdir)
    acc = np.zeros(H, np.float64)
    for c in range(NCORES):
        acc += res.results[c]["out"][0].astype(np.float64)
    out = acc.astype(np.float32).reshape(1, H, 1, 1)
    return out, res


def kernel(**inputs):
    out, _ = _run(inputs, trace=False)
    return out


def kernel_traced(tmpdir=None, **inputs):
    return _run(inputs, trace=True, tmpdir=tmpdir)
